# revision 61
# baseline (speedup 1.0000x reference)
"""Trainium2 Bass kernel for a dense cross-attention transformer block.

Reference computation (per batch b):
    xn = LN(x[b]); yn = LN(y[b])
    q = xn@Wq; k = yn@Wk; v = yn@Wv
    a = softmax(mask(q@k^T/sqrt(L)))
    x2 = xn + a@v; x3 = LN(x2)
    out1 = x3 + relu(x3@Win)@Wout
    returns (out1, yn)

Sharding: 8 cores = 4 batches x 2 halves. Core (b, h) handles query rows
[h*1024, (h+1)*1024) of batch b. With SPLIT_KV the core computes LN(y)/
k/v only for its own half of the key rows and a pair AllGather exchanges
(kT||v); otherwise each core computes k/v for all keys of its batch.

Precision: q/k production and the score matmul run in fp8 (float8e4,
DoubleRow perf mode, ~1.5x PE throughput; HW-verified rel err 5.4e-3 vs
2e-2 budget); v/attention-output/FFN matmuls are bf16 (fp8 FFN measured
3.5e-2 — over budget). All accumulation is f32 PSUM; LN/softmax stats
f32. Softmax skips the row-max subtraction (scaled logits are ~N(0,1),
exp can't overflow f32; masked entries give exp(-1.5e28) = 0).

Scheduling: qT stays SBUF-resident; softmax is fused per-qt into the
last score block; one big SBUF pool's slots are reused across phases
(qT->hT, S->Wout_lo, v->Wout_hi) to fit the 208KB/partition budget;
weight-stream DMAs are issued from a different engine than their casts
so the load pipeline runs at DMA rate.
"""

import numpy as np
import sys

for _p in ("/opt/trn_rl_repo",):
    if _p not in sys.path:
        sys.path.insert(0, _p)

import concourse.bass as bass
import concourse.bacc as bacc
import concourse.mybir as mybir
import concourse.tile as tile
from concourse.bass_utils import run_bass_kernel_spmd
from concourse.masks import make_identity

P = 128
E = 1024          # embedding dim
L = 4096          # latent dim
SK = 2048         # key rows per batch
SKH = 1024        # key rows per core when SPLIT_KV
SQH = 1024        # query rows per core (half batch)
B = 4
NCORES = 8
EC = E // P       # 8  e-chunks
LC = L // P       # 32 l-chunks
KC = SK // P      # 16 k-chunks (global)
KCH = SKH // P    # 8  k-chunks (own half)
QT = SQH // P     # 8  q-tiles per core
NEG = -1.0e30
INV_SQRT_L = 1.0 / 64.0
KVROWS = L + SKH  # kv_loc rows when SPLIT_KV: kT [L, SKH] then v [SKH, E]

F32 = mybir.dt.float32
BF16 = mybir.dt.bfloat16
F8 = mybir.dt.float8e4
I32 = mybir.dt.int32

AF = mybir.ActivationFunctionType
OP = mybir.AluOpType

GROUPS = [[0, 1], [2, 3], [4, 5], [6, 7]]

# Key-split + pair AllGather dedups k/v compute (~136us PE/core) but the
# collective's barrier makes runs unstable under axon (mesh desyncs, huge
# timing variance) for no measured net gain (1013us split vs 1023us
# nosplit). Ship the collective-free variant.
SPLIT_KV = False

_CACHE = {}
PHASE_MARKS = []


def _layernorm_tile(nc, pool, out_ap, in_ap, eps_tile):
    """LN over the free dim (1024) of a [128, 1024] f32 tile."""
    stats = pool.tile([P, 2, 6], F32, tag="ln_stats")
    mv = pool.tile([P, 2], F32, tag="ln_mv")
    xr = in_ap.rearrange("p (s d) -> p s d", s=2)
    for s in range(2):
        nc.vector.bn_stats(out=stats[:, s, :], in_=xr[:, s, :])
    nc.vector.bn_aggr(out=mv[:], in_=stats[:])
    sd = pool.tile([P, 1], F32, tag="ln_sd")
    nc.scalar.activation(out=sd[:], in_=mv[:, 1:2], func=AF.Sqrt, bias=eps_tile[:])
    rs = pool.tile([P, 1], F32, tag="ln_rs")
    nc.vector.reciprocal(out=rs[:], in_=sd[:])
    nc.vector.tensor_scalar(
        out=out_ap, in0=in_ap, scalar1=mv[:, 0:1], scalar2=rs[:],
        op0=OP.subtract, op1=OP.mult,
    )


def _build(phases="12vabf", sim=False, nocoll=False, split=None):
    if split is None:
        split = SPLIT_KV
    nc = bacc.Bacc("TRN2", target_bir_lowering=False, debug=False,
                   num_devices=1 if sim else NCORES)

    skr = SKH if split else SK
    x_h = nc.dram_tensor("x_h", [SQH, E], F32, kind="ExternalInput")
    y_h = nc.dram_tensor("y_h", [skr, E], F32, kind="ExternalInput")
    mask_h = nc.dram_tensor("mask_h", [SQH, SK], I32, kind="ExternalInput")
    Wq = nc.dram_tensor("Wq", [E, L], F32, kind="ExternalInput")
    Wk = nc.dram_tensor("Wk", [E, L], F32, kind="ExternalInput")
    Wv = nc.dram_tensor("Wv", [E, E], F32, kind="ExternalInput")
    Win = nc.dram_tensor("Win", [E, L], F32, kind="ExternalInput")
    Wout = nc.dram_tensor("Wout", [L, E], F32, kind="ExternalInput")

    out1 = nc.dram_tensor("out1", [SQH, E], F32, kind="ExternalOutput")
    yn_out = nc.dram_tensor("yn_out", [skr, E], F32, kind="ExternalOutput")

    if split:
        kv_loc = nc.dram_tensor("k_loc", [L, SKH], F8)
        kv_g = nc.dram_tensor("k_g", [2, L, SKH], F8)
        v_d = nc.dram_tensor("v_loc", [SKH, E], BF16)
        v_g = nc.dram_tensor("v_g", [2, SKH, E], BF16)
    else:
        kv_loc = nc.dram_tensor("kT_d", [L, SK], F8)
        kv_g = None
        v_d = nc.dram_tensor("v_d", [SK, E], BF16)
        v_g = None
    xn_d = nc.dram_tensor("xn_d", [SQH, E], F32)
    x3_d = nc.dram_tensor("x3_d", [SQH, E], F32)
    x3T_d = nc.dram_tensor("x3T_d", [E, SQH], BF16)

    with tile.TileContext(nc) as tc:
        _graph(nc, tc, x_h, y_h, mask_h, Wq, Wk, Wv, Win, Wout,
               out1, yn_out, kv_loc, kv_g, v_d, v_g, xn_d, x3_d, x3T_d,
               phases, sim or nocoll, split)
    nc.compile()
    return nc


def _graph(nc, tc, x_h, y_h, mask_h, Wq, Wk, Wv, Win, Wout,
           out1, yn_out, kv_loc, kv_g, v_d, v_g, xn_d, x3_d, x3T_d,
           phases, nocoll, split):

    PHASE_MARKS.clear()

    def mark(name):
        PHASE_MARKS.append((name, nc.next_id()))

    with tc.tile_pool(name="consts", bufs=1) as consts:
        ident = consts.tile([P, P], BF16)
        make_identity(nc, ident[:])
        eps_t = consts.tile([P, 1], F32)
        nc.vector.memset(eps_t[:], 1e-5)
        riall = consts.tile([P, QT], F32)   # softmax 1/rowsum, phases A->B

        mark("Pyk")
        _phase_yk(nc, tc, y_h, Wk, Wv, yn_out, kv_loc, kv_g, v_d, v_g,
                  ident, eps_t, phases, nocoll, split)

        # attn pool: slots reused across phases to fit SBUF.
        #   t64:  qT (q..A)      -> hT      (F)
        #   t32a: S  (A..B)      -> wout_lo (F)
        #   t32b: v_sb (A..B)    -> wout_hi (F)
        attn = tc.alloc_tile_pool(name="attn", bufs=1)
        qT = attn.tile([P, LC, SQH], F8, tag="t64")

        if "2" in phases:
            mark("Q")
            _phase_q(nc, tc, x_h, Wq, xn_d, qT, ident, eps_t)

        S = attn.tile([P, QT, SK], BF16, tag="t32a")   # [q_loc, qt, k]
        v_sb = attn.tile([P, KC, E], BF16, tag="t32b")

        if "a" in phases:
            mark("A")
            _phase_a(nc, tc, mask_h, kv_loc, kv_g, v_d, v_g, S, qT, riall,
                     v_sb, split)
        if "b" in phases and "a" in phases:
            mark("B")
            _phase_b(nc, tc, xn_d, x3_d, x3T_d, S, v_sb, riall,
                     ident, eps_t)
        if "f" in phases and "b" in phases and "a" in phases:
            mark("F")
            hT = attn.tile([P, LC, SQH], BF16, tag="t64")
            wout_lo = attn.tile([P, KC, E], BF16, tag="t32a")
            wout_hi = attn.tile([P, KC, E], BF16, tag="t32b")
            _phase_f(nc, tc, Win, Wout, x3_d, x3T_d, out1,
                     hT, wout_lo, wout_hi)
        attn.release()


def _phase_yk(nc, tc, y_h, Wk, Wv, yn_out, kv_loc, kv_g, v_d, v_g,
              ident, eps_t, phases, nocoll, split):
    """LN(y) -> ynT; v = yn@Wv -> spill; kT = Wk^T@ynT -> spill; gather."""
    nky = KCH if split else KC          # y row tiles
    nkc = (SKH if split else SK) // 512  # kT column chunks per lt
    with tc.tile_pool(name="ynT", bufs=1) as ynT_pool, \
         tc.tile_pool(name="p1_in", bufs=4) as p1_in, \
         tc.tile_pool(name="p1_tmp", bufs=12) as p1_tmp, \
         tc.tile_pool(name="p1_bf", bufs=5) as p1_bf, \
         tc.tile_pool(name="p1_ps", bufs=4, space="PSUM") as p1_ps, \
         tc.tile_pool(name="p2_wv", bufs=1) as p2_wv, \
         tc.tile_pool(name="p2_wk8", bufs=1) as p2_wk8, \
         tc.tile_pool(name="p2_ws", bufs=2) as p2_ws, \
         tc.tile_pool(name="p2_o", bufs=3) as p2_o, \
         tc.tile_pool(name="p2_ps", bufs=4, space="PSUM") as p2_ps:

        ynT = ynT_pool.tile([P, EC, nky * P], BF16)
        ynT8 = ynT_pool.tile([P, EC, nky * P], F8)   # fp8 twin for kT

        def ln_row_tile(src_t, row0, ntile_dst, dst_col0, spill_dst,
                        ntile8_dst=None):
            t_in = p1_in.tile([P, E], F32, tag="ln_in")
            nc.scalar.dma_start(out=t_in[:], in_=src_t[row0:row0 + P, :])
            t_n = p1_in.tile([P, E], F32, tag="ln_out")
            _layernorm_tile(nc, p1_tmp, t_n[:], t_in[:], eps_t)
            nc.gpsimd.dma_start(out=spill_dst[row0:row0 + P, :], in_=t_n[:])
            t_bf = p1_bf.tile([P, E], BF16, tag="ln_bf")
            nc.gpsimd.tensor_copy(out=t_bf[:], in_=t_n[:])
            for ec in range(EC):
                ps = p1_ps.tile([P, P], BF16, tag="tp")
                nc.tensor.transpose(
                    ps[:], t_bf[:, ec * P:(ec + 1) * P], ident[:])
                nc.scalar.copy(
                    out=ntile_dst[:, ec, dst_col0:dst_col0 + P], in_=ps[:])
                if ntile8_dst is not None:
                    # psum-reading copy: only DVE/Act may touch PSUM
                    nc.scalar.copy(
                        out=ntile8_dst[:, ec, dst_col0:dst_col0 + P],
                        in_=ps[:])

        wv_r = Wv.ap().rearrange("(c p) e -> p c e", p=P)
        wv_b = p2_wv.tile([P, EC, E], BF16)   # 2 MB resident
        v_w = v_d.ap().rearrange("(t p) e -> t p e", p=P)

        # one-shot Wv load first so the v chains aren't gated ~25us in
        wvc = p2_wv.tile([P, EC, E], F32, tag="wvc_full")
        nc.sync.dma_start(out=wvc[:], in_=wv_r[:])
        for ec in range(EC):
            nc.gpsimd.tensor_copy(out=wv_b[:, ec, :], in_=wvc[:, ec, :])

        # Wk via contiguous row-blocks (16KB/partition DMA lines — the
        # strided "(c p) l" gathers measured ~5x slower on HW) into
        # resident fp8 pair-tiles consumed directly by the DoubleRow
        # chains. Loads overlap the whole y-section.
        wk_rr = Wk.ap().rearrange("(c p) l -> c p l", p=P)
        wk8s = []
        for i in range(EC // 2):
            wt = p2_wk8.tile([P, 2, L], F8, tag=f"wk8_{i}")
            for j in range(2):
                for h2 in range(2):
                    csl = slice(h2 * (L // 2), (h2 + 1) * (L // 2))
                    wst = p2_ws.tile([P, L // 2], F32, tag="wst")
                    nc.sync.dma_start(out=wst[:], in_=wk_rr[2 * i + j, :, csl])
                    nc.scalar.copy(
                        out=wt[:, j, h2 * (L // 2):h2 * (L // 2) + L // 4],
                        in_=wst[:, :L // 4])
                    nc.vector.tensor_copy(
                        out=wt[:, j, h2 * (L // 2) + L // 4:(h2 + 1) * (L // 2)],
                        in_=wst[:, L // 4:])
            wk8s.append(wt)

        # ---- y tiles: LN + v matmuls interleaved ----
        for t in range(nky):
            ln_row_tile(y_h.ap(), t * P, ynT, t * P, yn_out.ap(), ynT8)
            if "v" in phases:
                for eo in range(E // 512):
                    ps = p2_ps.tile([P, 512], F32, tag="mm")
                    for ec in range(EC):
                        nc.tensor.matmul(
                            ps[:], ynT[:, ec, t * P:(t + 1) * P],
                            wv_b[:, ec, eo * 512:(eo + 1) * 512],
                            start=(ec == 0), stop=(ec == EC - 1))
                    vbf = p2_o.tile([P, 512], BF16, tag="vbf")
                    nc.vector.tensor_copy(out=vbf[:], in_=ps[:])
                    # scalar queue: sync is busy streaming Wk row-blocks
                    nc.scalar.dma_start(
                        out=v_w[t, :, eo * 512:(eo + 1) * 512], in_=vbf[:])

        if split and "v" in phases:
            # v exchange first: 2 MB, fully hidden under the kT loop
            if nocoll:
                nc.gpsimd.dma_start(out=v_g.ap()[0], in_=v_d.ap())
                nc.gpsimd.dma_start(out=v_g.ap()[1], in_=v_d.ap())
            else:
                nc.gpsimd.collective_compute(
                    "AllGather", mybir.AluOpType.bypass,
                    replica_groups=GROUPS,
                    ins=[v_d.ap()], outs=[v_g.ap()],
                )

        # ---- kT ----
        if "2" in phases:
            for lt in range(LC):
                lsl = slice(lt * P, (lt + 1) * P)
                # collect the full kT row in SBUF, spill with one DMA of
                # 2KB/partition lines instead of 4 writes of 512B lines
                kbf_row = p2_o.tile([P, nkc * 512], F8, tag="kbfrow")
                for kc in range(nkc):
                    ps = p2_ps.tile([P, 512], F32, tag="mm")
                    for ec in range(EC // 2):
                        nc.tensor.matmul(
                            ps[:], wk8s[ec][:, :, lsl],
                            ynT8[:, 2 * ec:2 * ec + 2, kc * 512:(kc + 1) * 512],
                            perf_mode=mybir.MatmulPerfMode.DoubleRow,
                            start=(ec == 0), stop=(ec == EC // 2 - 1))
                    nc.vector.tensor_copy(
                        out=kbf_row[:, kc * 512:(kc + 1) * 512], in_=ps[:])
                nc.scalar.dma_start(out=kv_loc.ap()[lsl, :], in_=kbf_row[:])

            if split:
                # ---- pair AllGather of kT (fp8) ----
                if nocoll:
                    nc.gpsimd.dma_start(out=kv_g.ap()[0], in_=kv_loc.ap())
                    nc.gpsimd.dma_start(out=kv_g.ap()[1], in_=kv_loc.ap())
                else:
                    nc.gpsimd.collective_compute(
                        "AllGather", mybir.AluOpType.bypass,
                        replica_groups=GROUPS,
                        ins=[kv_loc.ap()], outs=[kv_g.ap()],
                    )


def _phase_q(nc, tc, x_h, Wq, xn_d, qT, ident, eps_t):
    """LN(x) -> xnT; qT = Wq^T@xnT into resident SBUF tile."""
    with tc.tile_pool(name="xnT_pool", bufs=1) as xnT_pool, \
         tc.tile_pool(name="q1_in", bufs=2) as q1_in, \
         tc.tile_pool(name="q1_tmp", bufs=8) as q1_tmp, \
         tc.tile_pool(name="q1_bf", bufs=2) as q1_bf, \
         tc.tile_pool(name="q1_ps", bufs=4, space="PSUM") as q1_ps, \
         tc.tile_pool(name="pq_wk8", bufs=1) as pq_wk8, \
         tc.tile_pool(name="pq_ws", bufs=2) as pq_ws, \
         tc.tile_pool(name="pq_ps", bufs=4, space="PSUM") as pq_ps:

        xnT = xnT_pool.tile([P, EC, SQH], F8)   # 1 MB, q-production only

        # Wq row-blocks -> resident fp8 pair-tiles (see Wk note above)
        wq_rr = Wq.ap().rearrange("(c p) l -> c p l", p=P)
        wq8s = []
        for i in range(EC // 2):
            wt = pq_wk8.tile([P, 2, L], F8, tag=f"wq8_{i}")
            for j in range(2):
                for h2 in range(2):
                    csl = slice(h2 * (L // 2), (h2 + 1) * (L // 2))
                    wst = pq_ws.tile([P, L // 2], F32, tag="wst")
                    nc.sync.dma_start(out=wst[:], in_=wq_rr[2 * i + j, :, csl])
                    nc.scalar.copy(
                        out=wt[:, j, h2 * (L // 2):h2 * (L // 2) + L // 4],
                        in_=wst[:, :L // 4])
                    nc.vector.tensor_copy(
                        out=wt[:, j, h2 * (L // 2) + L // 4:(h2 + 1) * (L // 2)],
                        in_=wst[:, L // 4:])
            wq8s.append(wt)

        for t in range(QT):
            t_in = q1_in.tile([P, E], F32, tag="ln_in")
            nc.scalar.dma_start(out=t_in[:], in_=x_h.ap()[t * P:(t + 1) * P, :])
            t_n = q1_in.tile([P, E], F32, tag="ln_out")
            _layernorm_tile(nc, q1_tmp, t_n[:], t_in[:], eps_t)
            # scalar/vector: gpsimd is occupied by the kv AllGather here
            nc.scalar.dma_start(out=xn_d.ap()[t * P:(t + 1) * P, :], in_=t_n[:])
            t_bf = q1_bf.tile([P, E], BF16, tag="ln_bf")
            nc.vector.tensor_copy(out=t_bf[:], in_=t_n[:])
            for ec in range(EC):
                ps = q1_ps.tile([P, P], BF16, tag="tp")
                nc.tensor.transpose(
                    ps[:], t_bf[:, ec * P:(ec + 1) * P], ident[:])
                nc.scalar.copy(
                    out=xnT[:, ec, t * P:(t + 1) * P], in_=ps[:])

        for lt in range(LC):
            lsl = slice(lt * P, (lt + 1) * P)
            for qc in range(SQH // 512):
                ps = pq_ps.tile([P, 512], F32, tag="mm")
                for ec in range(EC // 2):
                    nc.tensor.matmul(
                        ps[:], wq8s[ec][:, :, lsl],
                        xnT[:, 2 * ec:2 * ec + 2, qc * 512:(qc + 1) * 512],
                        perf_mode=mybir.MatmulPerfMode.DoubleRow,
                        start=(ec == 0), stop=(ec == EC // 2 - 1))
                nc.vector.tensor_copy(
                    out=qT[:, lt, qc * 512:(qc + 1) * 512], in_=ps[:])


def _phase_a(nc, tc, mask_h, kv_loc, kv_g, v_d, v_g, S, qT, riall,
             v_sb, split):
    """Scores (fp8 DoubleRow) + mask + fused per-qt softmax."""
    # 1024-key blocks: 1KB DMA lines and half the descriptors vs 512
    BW = 512 if split else 1024
    NKB = SK // BW
    NSUB = BW // 512
    with tc.tile_pool(name="pa_kt", bufs=2) as pa_kt, \
         tc.tile_pool(name="pa_mi", bufs=2) as pa_mi, \
         tc.tile_pool(name="pa_mf", bufs=2) as pa_mf, \
         tc.tile_pool(name="pa_sm", bufs=4) as pa_sm, \
         tc.tile_pool(name="pa_ps", bufs=3, space="PSUM") as pa_ps:

        for kb in range(NKB):
            kt_blk = pa_kt.tile([P, LC, BW], F8, tag="ktb")
            if split:
                g, sub = kb // 2, kb % 2
                kt_src = kv_g.ap()[g].rearrange(
                    "(c p) k -> p c k", p=P)[:, :, sub * 512:(sub + 1) * 512]
            else:
                kt_src = kv_loc.ap().rearrange(
                    "(c p) k -> p c k", p=P)[:, :, kb * BW:(kb + 1) * BW]
            # gpsimd queue: keeps the scalar queue free for the Wq stream
            # and avoids head-blocking it behind the collective-gated load
            nc.gpsimd.dma_start(out=kt_blk[:], in_=kt_src)
            if kb == 0:
                # prefetch v for phase B
                if split:
                    for gv in range(2):
                        nc.gpsimd.dma_start(
                            out=v_sb[:, gv * KCH:(gv + 1) * KCH, :],
                            in_=v_g.ap()[gv].rearrange(
                                "(c p) e -> p c e", p=P))
                else:
                    nc.gpsimd.dma_start(
                        out=v_sb[:],
                        in_=v_d.ap().rearrange("(c p) e -> p c e", p=P))
            for sub in range(NSUB):
                ksl = slice(kb * BW + sub * 512, kb * BW + (sub + 1) * 512)
                last = (kb == NKB - 1) and (sub == NSUB - 1)
                for qt in range(QT):
                    ps = pa_ps.tile([P, 512], F32, tag="s")
                    for lc in range(LC // 2):
                        nc.tensor.matmul(
                            ps[:],
                            qT[:, 2 * lc:2 * lc + 2, qt * P:(qt + 1) * P],
                            kt_blk[:, 2 * lc:2 * lc + 2,
                                   sub * 512:(sub + 1) * 512],
                            perf_mode=mybir.MatmulPerfMode.DoubleRow,
                            start=(lc == 0), stop=(lc == LC // 2 - 1))
                    mi = pa_mi.tile([P, 512], I32, tag="mi")
                    nc.sync.dma_start(
                        out=mi[:], in_=mask_h.ap()[qt * P:(qt + 1) * P, ksl])
                    mf = pa_mf.tile([P, 512], F32, tag="mf")
                    nc.vector.tensor_scalar_mul(
                        out=mf[:], in0=mi[:], scalar1=NEG)
                    nc.vector.tensor_add(
                        out=S[:, qt, ksl], in0=ps[:], in1=mf[:])
                    if last:
                        # No row-max subtraction: scaled logits are ~N(0,1)
                        # (|s|/64 < ~6), exp can't overflow f32; masked
                        # entries give exp(-1.5e28) = 0.
                        rs = pa_sm.tile([P, 1], F32, tag="rs")
                        nc.scalar.activation(
                            out=S[:, qt, :], in_=S[:, qt, :], func=AF.Exp,
                            bias=0.0, scale=INV_SQRT_L, accum_out=rs[:])
                        nc.vector.reciprocal(
                            out=riall[:, qt:qt + 1], in_=rs[:])


def _phase_b(nc, tc, xn_d, x3_d, x3T_d, S, v_sb, riall, ident, eps_t):
    """P^T, out2 = P@V, residual, LN3, x3T spill (transposed, bf16)."""
    x3T_w = x3T_d.ap().rearrange("(c p) q -> p c q", p=P)
    with tc.tile_pool(name="pb_pt", bufs=2 * KC) as pb_pt, \
         tc.tile_pool(name="pb_x", bufs=2) as pb_x, \
         tc.tile_pool(name="x3b_pool", bufs=QT) as x3b_pool, \
         tc.tile_pool(name="pb_st", bufs=2) as pb_st, \
         tc.tile_pool(name="pb_tmp", bufs=4) as pb_tmp, \
         tc.tile_pool(name="pb_ptps", bufs=4, space="PSUM") as pb_ptps, \
         tc.tile_pool(name="pb_ps", bufs=4, space="PSUM") as pb_ps:

        def transpose_s(qt):
            pts = []
            for kc in range(KC):
                pps = pb_ptps.tile([P, P], BF16, tag="ptps")
                nc.tensor.transpose(
                    pps[:], S[:, qt, kc * P:(kc + 1) * P], ident[:])
                pt = pb_pt.tile([P, P], BF16, tag="pt")
                # alternate copy engines so the a@v chain isn't gated on DVE
                eng = nc.vector if kc % 2 == 0 else nc.scalar
                if eng is nc.scalar:
                    nc.scalar.copy(out=pt[:], in_=pps[:])
                else:
                    nc.vector.tensor_copy(out=pt[:], in_=pps[:])
                pts.append(pt)
            return pts

        x3bs = []
        pts_next = transpose_s(0)
        for qt in range(QT):
            pts = pts_next
            if qt + 1 < QT:
                pts_next = transpose_s(qt + 1)

            xn_t = pb_x.tile([P, E], F32, tag="xn")
            # gpsimd: the Act engine is busy with the softmax exp tail here
            nc.gpsimd.dma_start(
                out=xn_t[:], in_=xn_d.ap()[qt * P:(qt + 1) * P, :])
            x2 = pb_x.tile([P, E], F32, tag="x2")
            for eo in range(E // 512):
                ps = pb_ps.tile([P, 512], F32, tag="o")
                for kc in range(KC):
                    nc.tensor.matmul(
                        ps[:], pts[kc][:],
                        v_sb[:, kc, eo * 512:(eo + 1) * 512],
                        start=(kc == 0), stop=(kc == KC - 1))
                nc.vector.tensor_scalar_mul(
                    out=x2[:, eo * 512:(eo + 1) * 512], in0=ps[:],
                    scalar1=riall[:, qt:qt + 1])
            nc.vector.tensor_add(out=x2[:], in0=x2[:], in1=xn_t[:])

            x3 = pb_x.tile([P, E], F32, tag="x3")
            _layernorm_tile(nc, pb_tmp, x3[:], x2[:], eps_t)
            nc.gpsimd.dma_start(
                out=x3_d.ap()[qt * P:(qt + 1) * P, :], in_=x3[:])
            x3b = x3b_pool.tile([P, E], BF16, tag="x3b")
            nc.gpsimd.tensor_copy(out=x3b[:], in_=x3[:])
            x3bs.append(x3b)

        for qt in range(QT):    # trailing transposes: no PE head-of-line
            st = pb_st.tile([P, EC, P], BF16, tag="st")
            for ec in range(EC):
                pps = pb_ptps.tile([P, P], BF16, tag="ptps")
                nc.tensor.transpose(
                    pps[:], x3bs[qt][:, ec * P:(ec + 1) * P], ident[:])
                nc.scalar.copy(out=st[:, ec, :], in_=pps[:])
            nc.sync.dma_start(
                out=x3T_w[:, :, qt * P:(qt + 1) * P], in_=st[:])


def _phase_f(nc, tc, Win, Wout, x3_d, x3T_d, out1, hT, wout_lo, wout_hi):
    """FFN: hT = relu(Win^T @ x3T); out = hT^T @ Wout + x3."""
    wout_r = Wout.ap().rearrange("(c p) e -> p c e", p=P)
    with tc.tile_pool(name="pf_x3T", bufs=1) as pf_x3T, \
         tc.tile_pool(name="pf_w", bufs=3) as pf_w, \
         tc.tile_pool(name="pf_wf", bufs=3) as pf_wf, \
         tc.tile_pool(name="pf_wb", bufs=3) as pf_wb, \
         tc.tile_pool(name="pf_x", bufs=2) as pf_x, \
         tc.tile_pool(name="pf_o", bufs=3) as pf_o, \
         tc.tile_pool(name="pf_ps", bufs=3, space="PSUM") as pf_ps:

        x3T = pf_x3T.tile([P, EC, SQH], BF16)    # 2 MB
        nc.scalar.dma_start(
            out=x3T[:], in_=x3T_d.ap().rearrange("(c p) q -> p c q", p=P))

        for lt in range(LC):
            lsl = slice(lt * P, (lt + 1) * P)
            wi_f = pf_w.tile([P, EC, P], F32, tag="wi_f")
            nc.scalar.dma_start(
                out=wi_f[:],
                in_=Win.ap()[:, lsl].rearrange("(c p) l -> p c l", p=P))
            wi_b = pf_wb.tile([P, EC, P], BF16, tag="wi_b")
            nc.scalar.copy(out=wi_b[:], in_=wi_f[:])
            # Wout loads on the gpsimd queue, cast on DVE: keeps the
            # scalar queue free for Win and the Act engine for relu copies
            wf = pf_wf.tile([P, E], F32, tag="wo_f")
            nc.gpsimd.dma_start(out=wf[:], in_=wout_r[:, lt, :])
            wdst = wout_lo if lt < KC else wout_hi
            nc.vector.tensor_copy(out=wdst[:, lt % KC, :], in_=wf[:])
            for qc in range(SQH // 512):
                ps = pf_ps.tile([P, 512], F32, tag="h")
                for ec in range(EC):
                    nc.tensor.matmul(
                        ps[:], wi_b[:, ec, :],
                        x3T[:, ec, qc * 512:(qc + 1) * 512],
                        start=(ec == 0), stop=(ec == EC - 1))
                nc.scalar.activation(
                    out=hT[:, lt, qc * 512:(qc + 1) * 512], in_=ps[:],
                    func=AF.Relu)

        for qt in range(QT):
            x3_t = pf_x.tile([P, E], F32, tag="x3r")
            nc.sync.dma_start(
                out=x3_t[:], in_=x3_d.ap()[qt * P:(qt + 1) * P, :])
            for eo in range(E // 512):
                ps = pf_ps.tile([P, 512], F32, tag="f")
                for lc in range(LC):
                    wsrc = wout_lo if lc < KC else wout_hi
                    nc.tensor.matmul(
                        ps[:], hT[:, lc, qt * P:(qt + 1) * P],
                        wsrc[:, lc % KC, eo * 512:(eo + 1) * 512],
                        start=(lc == 0), stop=(lc == LC - 1))
                o_t = pf_o.tile([P, 512], F32, tag="o")
                nc.vector.tensor_add(
                    out=o_t[:], in0=ps[:], in1=x3_t[:, eo * 512:(eo + 1) * 512])
                nc.sync.dma_start(
                    out=out1.ap()[qt * P:(qt + 1) * P, eo * 512:(eo + 1) * 512],
                    in_=o_t[:])


def _get_compiled(phases="12vabf", sim=False, nocoll=False, split=None):
    key = (phases, sim, nocoll, SPLIT_KV if split is None else split)
    if key not in _CACHE:
        _CACHE[key] = _build(phases, sim, nocoll, split)
    return _CACHE[key]


def _check_trivial(inputs):
    for n in ("ln1_w", "ln2_w", "ln3_w"):
        if n in inputs and not np.allclose(np.asarray(inputs[n]), 1.0):
            raise NotImplementedError(f"nontrivial {n} unsupported")
    for n in ("ln1_b", "ln2_b", "ln3_b", "bq", "bk", "bv", "bin", "bout"):
        if n in inputs and not np.allclose(np.asarray(inputs[n]), 0.0):
            raise NotImplementedError(f"nontrivial {n} unsupported")


LAST_EXEC_NS = None
TRACE = False


def make_in_maps(x, y, mask, Wq, Wk, Wv, Win, Wout, split=None):
    if split is None:
        split = SPLIT_KV
    in_maps = []
    for c in range(NCORES):
        b, h = c // 2, c % 2
        ysl = y[b, h * SKH:(h + 1) * SKH] if split else y[b]
        in_maps.append({
            "x_h": np.ascontiguousarray(x[b, h * SQH:(h + 1) * SQH]),
            "y_h": np.ascontiguousarray(ysl),
            "mask_h": np.ascontiguousarray(mask[b, h * SQH:(h + 1) * SQH]),
            "Wq": Wq, "Wk": Wk, "Wv": Wv, "Win": Win, "Wout": Wout,
        })
    return in_maps


def kernel(**inputs):
    global LAST_EXEC_NS
    _check_trivial(inputs)
    x = np.ascontiguousarray(np.asarray(inputs["x"], dtype=np.float32))
    y = np.ascontiguousarray(np.asarray(inputs["y"], dtype=np.float32))
    mask = np.ascontiguousarray(np.asarray(inputs["mask"], dtype=np.int32))
    Wq = np.ascontiguousarray(np.asarray(inputs["Wq"], dtype=np.float32))
    Wk = np.ascontiguousarray(np.asarray(inputs["Wk"], dtype=np.float32))
    Wv = np.ascontiguousarray(np.asarray(inputs["Wv"], dtype=np.float32))
    Win = np.ascontiguousarray(np.asarray(inputs["Win"], dtype=np.float32))
    Wout = np.ascontiguousarray(np.asarray(inputs["Wout"], dtype=np.float32))

    nc = _get_compiled()
    in_maps = make_in_maps(x, y, mask, Wq, Wk, Wv, Win, Wout)
    last_err = None
    for attempt in range(3):
        try:
            res = run_bass_kernel_spmd(nc, in_maps,
                                       core_ids=list(range(NCORES)),
                                       trace=TRACE)
            break
        except Exception as e:   # transient device/terminal errors
            last_err = e
            import time as _time
            _time.sleep(10)
    else:
        raise last_err
    LAST_EXEC_NS = res.exec_time_ns
    outs = res.results
    o1 = np.empty((B, 2 * SQH, E), np.float32)
    yn = np.empty((B, SK, E), np.float32)
    for c in range(NCORES):
        b, h = c // 2, c % 2
        o1[b, h * SQH:(h + 1) * SQH] = outs[c]["out1"]
        if SPLIT_KV:
            yn[b, h * SKH:(h + 1) * SKH] = outs[c]["yn_out"]
        elif h == 0:
            yn[b] = outs[c]["yn_out"]
    return o1, yn


# revision 63
# speedup vs baseline: 1.2361x; 1.2361x over previous
"""Trainium2 Bass kernel for a dense cross-attention transformer block.

Reference computation (per batch b):
    xn = LN(x[b]); yn = LN(y[b])
    q = xn@Wq; k = yn@Wk; v = yn@Wv
    a = softmax(mask(q@k^T/sqrt(L)))
    x2 = xn + a@v; x3 = LN(x2)
    out1 = x3 + relu(x3@Win)@Wout
    returns (out1, yn)

Sharding: 8 cores = 4 batches x 2 halves. Core (b, h) handles query rows
[h*1024, (h+1)*1024) of batch b. With SPLIT_KV the core computes LN(y)/
k/v only for its own half of the key rows and a pair AllGather exchanges
(kT||v); otherwise each core computes k/v for all keys of its batch.

Precision: q/k production and the score matmul run in fp8 (float8e4,
DoubleRow perf mode, ~1.5x PE throughput; HW-verified rel err 5.4e-3 vs
2e-2 budget); v/attention-output/FFN matmuls are bf16 (fp8 FFN measured
3.5e-2 — over budget). All accumulation is f32 PSUM; LN/softmax stats
f32. Softmax skips the row-max subtraction (scaled logits are ~N(0,1),
exp can't overflow f32; masked entries give exp(-1.5e28) = 0).

Scheduling: qT stays SBUF-resident; softmax is fused per-qt into the
last score block; one big SBUF pool's slots are reused across phases
(qT->hT, S->Wout_lo, v->Wout_hi) to fit the 208KB/partition budget;
weight-stream DMAs are issued from a different engine than their casts
so the load pipeline runs at DMA rate.
"""

import numpy as np
import sys

for _p in ("/opt/trn_rl_repo",):
    if _p not in sys.path:
        sys.path.insert(0, _p)

import concourse.bass as bass
import concourse.bacc as bacc
import concourse.mybir as mybir
import concourse.tile as tile
from concourse.bass_utils import run_bass_kernel_spmd
from concourse.masks import make_identity

P = 128
E = 1024          # embedding dim
L = 4096          # latent dim
SK = 2048         # key rows per batch
SKH = 1024        # key rows per core when SPLIT_KV
SQH = 1024        # query rows per core (half batch)
B = 4
NCORES = 8
EC = E // P       # 8  e-chunks
LC = L // P       # 32 l-chunks
KC = SK // P      # 16 k-chunks (global)
KCH = SKH // P    # 8  k-chunks (own half)
QT = SQH // P     # 8  q-tiles per core
NEG = -1.0e30
INV_SQRT_L = 1.0 / 64.0
KVROWS = L + SKH  # kv_loc rows when SPLIT_KV: kT [L, SKH] then v [SKH, E]

F32 = mybir.dt.float32
BF16 = mybir.dt.bfloat16
F8 = mybir.dt.float8e4
I32 = mybir.dt.int32

AF = mybir.ActivationFunctionType
OP = mybir.AluOpType

GROUPS = [[0, 1], [2, 3], [4, 5], [6, 7]]

# Key-split + pair AllGather dedups k/v compute (~136us PE/core) but the
# collective's barrier makes runs unstable under axon (mesh desyncs, huge
# timing variance) for no measured net gain (1013us split vs 1023us
# nosplit). Ship the collective-free variant.
SPLIT_KV = False

_CACHE = {}
PHASE_MARKS = []


def _layernorm_tile(nc, pool, out_ap, in_ap, eps_tile):
    """LN over the free dim (1024) of a [128, 1024] f32 tile."""
    stats = pool.tile([P, 2, 6], F32, tag="ln_stats")
    mv = pool.tile([P, 2], F32, tag="ln_mv")
    xr = in_ap.rearrange("p (s d) -> p s d", s=2)
    for s in range(2):
        nc.vector.bn_stats(out=stats[:, s, :], in_=xr[:, s, :])
    nc.vector.bn_aggr(out=mv[:], in_=stats[:])
    sd = pool.tile([P, 1], F32, tag="ln_sd")
    nc.scalar.activation(out=sd[:], in_=mv[:, 1:2], func=AF.Sqrt, bias=eps_tile[:])
    rs = pool.tile([P, 1], F32, tag="ln_rs")
    nc.vector.reciprocal(out=rs[:], in_=sd[:])
    nc.vector.tensor_scalar(
        out=out_ap, in0=in_ap, scalar1=mv[:, 0:1], scalar2=rs[:],
        op0=OP.subtract, op1=OP.mult,
    )


def _build(phases="12vabf", sim=False, nocoll=False, split=None):
    if split is None:
        split = SPLIT_KV
    nc = bacc.Bacc("TRN2", target_bir_lowering=False, debug=False,
                   num_devices=1 if sim else NCORES)

    skr = SKH if split else SK
    x_h = nc.dram_tensor("x_h", [SQH, E], F32, kind="ExternalInput")
    y_h = nc.dram_tensor("y_h", [skr, E], F32, kind="ExternalInput")
    mask_h = nc.dram_tensor("mask_h", [SQH, SK], I32, kind="ExternalInput")
    Wq = nc.dram_tensor("Wq", [E, L], F32, kind="ExternalInput")
    Wk = nc.dram_tensor("Wk", [E, L], F32, kind="ExternalInput")
    Wv = nc.dram_tensor("Wv", [E, E], F32, kind="ExternalInput")
    Win = nc.dram_tensor("Win", [E, L], F32, kind="ExternalInput")
    Wout = nc.dram_tensor("Wout", [L, E], F32, kind="ExternalInput")

    out1 = nc.dram_tensor("out1", [SQH, E], F32, kind="ExternalOutput")
    yn_out = nc.dram_tensor("yn_out", [skr, E], F32, kind="ExternalOutput")

    if split:
        kv_loc = nc.dram_tensor("k_loc", [L, SKH], F8)
        kv_g = nc.dram_tensor("k_g", [2, L, SKH], F8)
        v_d = nc.dram_tensor("v_loc", [SKH, E], BF16)
        v_g = nc.dram_tensor("v_g", [2, SKH, E], BF16)
    else:
        kv_loc = nc.dram_tensor("kT_d", [L, SK], F8)
        kv_g = None
        v_d = nc.dram_tensor("v_d", [SK, E], BF16)
        v_g = None
    xn_d = nc.dram_tensor("xn_d", [SQH, E], F32)
    x3_d = nc.dram_tensor("x3_d", [SQH, E], F32)
    x3T_d = nc.dram_tensor("x3T_d", [E, SQH], BF16)

    with tile.TileContext(nc) as tc:
        _graph(nc, tc, x_h, y_h, mask_h, Wq, Wk, Wv, Win, Wout,
               out1, yn_out, kv_loc, kv_g, v_d, v_g, xn_d, x3_d, x3T_d,
               phases, sim or nocoll, split)
    nc.compile()
    return nc


def _graph(nc, tc, x_h, y_h, mask_h, Wq, Wk, Wv, Win, Wout,
           out1, yn_out, kv_loc, kv_g, v_d, v_g, xn_d, x3_d, x3T_d,
           phases, nocoll, split):

    PHASE_MARKS.clear()

    def mark(name):
        PHASE_MARKS.append((name, nc.next_id()))

    with tc.tile_pool(name="consts", bufs=1) as consts:
        ident = consts.tile([P, P], BF16)
        make_identity(nc, ident[:])
        eps_t = consts.tile([P, 1], F32)
        nc.vector.memset(eps_t[:], 1e-5)
        riall = consts.tile([P, QT], F32)   # softmax 1/rowsum, phases A->B

        mark("Pyk")
        _phase_yk(nc, tc, y_h, Wk, Wv, yn_out, kv_loc, kv_g, v_d, v_g,
                  ident, eps_t, phases, nocoll, split)

        # attn pool: slots reused across phases to fit SBUF.
        #   t64:  qT (q..A)      -> hT      (F)
        #   t32a: S  (A..B)      -> wout_lo (F)
        #   t32b: v_sb (A..B)    -> wout_hi (F)
        attn = tc.alloc_tile_pool(name="attn", bufs=1)
        qT = attn.tile([P, LC, SQH], F8, tag="t64")

        if "2" in phases:
            mark("Q")
            _phase_q(nc, tc, x_h, Wq, xn_d, qT, ident, eps_t)

        S = attn.tile([P, QT, SK], BF16, tag="t32a")   # [q_loc, qt, k]
        v_sb = attn.tile([P, KC, E], BF16, tag="t32b")

        if "a" in phases:
            mark("A")
            _phase_a(nc, tc, mask_h, kv_loc, kv_g, v_d, v_g, S, qT, riall,
                     v_sb, split)
        if "b" in phases and "a" in phases:
            mark("B")
            _phase_b(nc, tc, xn_d, x3_d, x3T_d, S, v_sb, riall,
                     ident, eps_t)
        if "f" in phases and "b" in phases and "a" in phases:
            mark("F")
            hT = attn.tile([P, LC, SQH], BF16, tag="t64")
            wout_lo = attn.tile([P, KC, E], BF16, tag="t32a")
            wout_hi = attn.tile([P, KC, E], BF16, tag="t32b")
            _phase_f(nc, tc, Win, Wout, x3_d, x3T_d, out1,
                     hT, wout_lo, wout_hi)
        attn.release()


def _phase_yk(nc, tc, y_h, Wk, Wv, yn_out, kv_loc, kv_g, v_d, v_g,
              ident, eps_t, phases, nocoll, split):
    """LN(y) -> ynT; v = yn@Wv -> spill; kT = Wk^T@ynT -> spill; gather."""
    nky = KCH if split else KC          # y row tiles
    nkc = (SKH if split else SK) // 512  # kT column chunks per lt
    with tc.tile_pool(name="ynT", bufs=1) as ynT_pool, \
         tc.tile_pool(name="p1_in", bufs=4) as p1_in, \
         tc.tile_pool(name="p1_tmp", bufs=12) as p1_tmp, \
         tc.tile_pool(name="p1_bf", bufs=5) as p1_bf, \
         tc.tile_pool(name="p1_ps", bufs=4, space="PSUM") as p1_ps, \
         tc.tile_pool(name="p2_wv", bufs=1) as p2_wv, \
         tc.tile_pool(name="p2_wk8", bufs=1) as p2_wk8, \
         tc.tile_pool(name="p2_ws", bufs=2) as p2_ws, \
         tc.tile_pool(name="p2_o", bufs=3) as p2_o, \
         tc.tile_pool(name="p2_ps", bufs=4, space="PSUM") as p2_ps:

        ynT = ynT_pool.tile([P, EC, nky * P], BF16)
        ynT8 = ynT_pool.tile([P, EC, nky * P], F8)   # fp8 twin for kT

        def ln_row_tile(src_t, row0, ntile_dst, dst_col0, spill_dst,
                        ntile8_dst=None):
            t_in = p1_in.tile([P, E], F32, tag="ln_in")
            nc.scalar.dma_start(out=t_in[:], in_=src_t[row0:row0 + P, :])
            t_n = p1_in.tile([P, E], F32, tag="ln_out")
            _layernorm_tile(nc, p1_tmp, t_n[:], t_in[:], eps_t)
            nc.gpsimd.dma_start(out=spill_dst[row0:row0 + P, :], in_=t_n[:])
            t_bf = p1_bf.tile([P, E], BF16, tag="ln_bf")
            nc.gpsimd.tensor_copy(out=t_bf[:], in_=t_n[:])
            for ec in range(EC):
                ps = p1_ps.tile([P, P], BF16, tag="tp")
                nc.tensor.transpose(
                    ps[:], t_bf[:, ec * P:(ec + 1) * P], ident[:])
                nc.scalar.copy(
                    out=ntile_dst[:, ec, dst_col0:dst_col0 + P], in_=ps[:])
                if ntile8_dst is not None:
                    # psum-reading copy: only DVE/Act may touch PSUM
                    nc.scalar.copy(
                        out=ntile8_dst[:, ec, dst_col0:dst_col0 + P],
                        in_=ps[:])

        wv_r = Wv.ap().rearrange("(c p) e -> p c e", p=P)
        wv_b = p2_wv.tile([P, EC, E], BF16)   # 2 MB resident
        v_w = v_d.ap().rearrange("(t p) e -> t p e", p=P)

        # one-shot Wv load first so the v chains aren't gated ~25us in
        wvc = p2_wv.tile([P, EC, E], F32, tag="wvc_full")
        nc.sync.dma_start(out=wvc[:], in_=wv_r[:])
        for ec in range(EC):
            nc.gpsimd.tensor_copy(out=wv_b[:, ec, :], in_=wvc[:, ec, :])

        # Wk via contiguous row-blocks (16KB/partition DMA lines — the
        # strided "(c p) l" gathers measured ~5x slower on HW) into
        # resident fp8 pair-tiles consumed directly by the DoubleRow
        # chains. Loads overlap the whole y-section.
        wk_rr = Wk.ap().rearrange("(c p) l -> c p l", p=P)
        wk8s = []
        for i in range(EC // 2):
            wt = p2_wk8.tile([P, 2, L], F8, tag=f"wk8_{i}")
            for j in range(2):
                for h2 in range(2):
                    csl = slice(h2 * (L // 2), (h2 + 1) * (L // 2))
                    wst = p2_ws.tile([P, L // 2], F32, tag="wst")
                    nc.sync.dma_start(out=wst[:], in_=wk_rr[2 * i + j, :, csl])
                    nc.scalar.copy(
                        out=wt[:, j, h2 * (L // 2):h2 * (L // 2) + L // 4],
                        in_=wst[:, :L // 4])
                    nc.vector.tensor_copy(
                        out=wt[:, j, h2 * (L // 2) + L // 4:(h2 + 1) * (L // 2)],
                        in_=wst[:, L // 4:])
            wk8s.append(wt)

        # ---- y tiles: LN + v matmuls interleaved ----
        for t in range(nky):
            ln_row_tile(y_h.ap(), t * P, ynT, t * P, yn_out.ap(), ynT8)
            if "v" in phases:
                for eo in range(E // 512):
                    ps = p2_ps.tile([P, 512], F32, tag="mm")
                    for ec in range(EC):
                        nc.tensor.matmul(
                            ps[:], ynT[:, ec, t * P:(t + 1) * P],
                            wv_b[:, ec, eo * 512:(eo + 1) * 512],
                            start=(ec == 0), stop=(ec == EC - 1))
                    vbf = p2_o.tile([P, 512], BF16, tag="vbf")
                    nc.vector.tensor_copy(out=vbf[:], in_=ps[:])
                    # scalar queue: sync is busy streaming Wk row-blocks
                    nc.scalar.dma_start(
                        out=v_w[t, :, eo * 512:(eo + 1) * 512], in_=vbf[:])

        if split and "v" in phases:
            # v exchange first: 2 MB, fully hidden under the kT loop
            if nocoll:
                nc.gpsimd.dma_start(out=v_g.ap()[0], in_=v_d.ap())
                nc.gpsimd.dma_start(out=v_g.ap()[1], in_=v_d.ap())
            else:
                nc.gpsimd.collective_compute(
                    "AllGather", mybir.AluOpType.bypass,
                    replica_groups=GROUPS,
                    ins=[v_d.ap()], outs=[v_g.ap()],
                )

        # ---- kT ----
        if "2" in phases:
            for lt in range(LC):
                lsl = slice(lt * P, (lt + 1) * P)
                # collect the full kT row in SBUF, spill with one DMA of
                # 2KB/partition lines instead of nkc writes of 512B lines
                kbf_row = p2_o.tile([P, nkc * 512], F8, tag="kbfrow")
                for kc in range(nkc):
                    ps = p2_ps.tile([P, 512], F32, tag="mm")
                    for ec in range(EC // 2):
                        nc.tensor.matmul(
                            ps[:], wk8s[ec][:, :, lsl],
                            ynT8[:, 2 * ec:2 * ec + 2, kc * 512:(kc + 1) * 512],
                            perf_mode=mybir.MatmulPerfMode.DoubleRow,
                            start=(ec == 0), stop=(ec == EC // 2 - 1))
                    nc.vector.tensor_copy(
                        out=kbf_row[:, kc * 512:(kc + 1) * 512], in_=ps[:])
                nc.scalar.dma_start(out=kv_loc.ap()[lsl, :], in_=kbf_row[:])

            if split:
                # ---- pair AllGather of kT (fp8) ----
                if nocoll:
                    nc.gpsimd.dma_start(out=kv_g.ap()[0], in_=kv_loc.ap())
                    nc.gpsimd.dma_start(out=kv_g.ap()[1], in_=kv_loc.ap())
                else:
                    nc.gpsimd.collective_compute(
                        "AllGather", mybir.AluOpType.bypass,
                        replica_groups=GROUPS,
                        ins=[kv_loc.ap()], outs=[kv_g.ap()],
                    )


def _phase_q(nc, tc, x_h, Wq, xn_d, qT, ident, eps_t):
    """LN(x) -> xnT; qT = Wq^T@xnT into resident SBUF tile."""
    with tc.tile_pool(name="xnT_pool", bufs=1) as xnT_pool, \
         tc.tile_pool(name="q1_in", bufs=2) as q1_in, \
         tc.tile_pool(name="q1_tmp", bufs=8) as q1_tmp, \
         tc.tile_pool(name="q1_bf", bufs=2) as q1_bf, \
         tc.tile_pool(name="q1_ps", bufs=4, space="PSUM") as q1_ps, \
         tc.tile_pool(name="pq_wk8", bufs=1) as pq_wk8, \
         tc.tile_pool(name="pq_ws", bufs=2) as pq_ws, \
         tc.tile_pool(name="pq_ps", bufs=4, space="PSUM") as pq_ps:

        xnT = xnT_pool.tile([P, EC, SQH], F8)   # 1 MB, q-production only

        # Wq row-blocks -> resident fp8 pair-tiles (see Wk note above)
        wq_rr = Wq.ap().rearrange("(c p) l -> c p l", p=P)
        wq8s = []
        for i in range(EC // 2):
            wt = pq_wk8.tile([P, 2, L], F8, tag=f"wq8_{i}")
            for j in range(2):
                for h2 in range(2):
                    csl = slice(h2 * (L // 2), (h2 + 1) * (L // 2))
                    wst = pq_ws.tile([P, L // 2], F32, tag="wst")
                    nc.sync.dma_start(out=wst[:], in_=wq_rr[2 * i + j, :, csl])
                    nc.scalar.copy(
                        out=wt[:, j, h2 * (L // 2):h2 * (L // 2) + L // 4],
                        in_=wst[:, :L // 4])
                    nc.vector.tensor_copy(
                        out=wt[:, j, h2 * (L // 2) + L // 4:(h2 + 1) * (L // 2)],
                        in_=wst[:, L // 4:])
            wq8s.append(wt)

        for t in range(QT):
            t_in = q1_in.tile([P, E], F32, tag="ln_in")
            nc.scalar.dma_start(out=t_in[:], in_=x_h.ap()[t * P:(t + 1) * P, :])
            t_n = q1_in.tile([P, E], F32, tag="ln_out")
            _layernorm_tile(nc, q1_tmp, t_n[:], t_in[:], eps_t)
            # scalar/vector: gpsimd is occupied by the kv AllGather here
            nc.scalar.dma_start(out=xn_d.ap()[t * P:(t + 1) * P, :], in_=t_n[:])
            t_bf = q1_bf.tile([P, E], BF16, tag="ln_bf")
            nc.vector.tensor_copy(out=t_bf[:], in_=t_n[:])
            for ec in range(EC):
                ps = q1_ps.tile([P, P], BF16, tag="tp")
                nc.tensor.transpose(
                    ps[:], t_bf[:, ec * P:(ec + 1) * P], ident[:])
                nc.scalar.copy(
                    out=xnT[:, ec, t * P:(t + 1) * P], in_=ps[:])

        for lt in range(LC):
            lsl = slice(lt * P, (lt + 1) * P)
            for qc in range(SQH // 512):
                ps = pq_ps.tile([P, 512], F32, tag="mm")
                for ec in range(EC // 2):
                    nc.tensor.matmul(
                        ps[:], wq8s[ec][:, :, lsl],
                        xnT[:, 2 * ec:2 * ec + 2, qc * 512:(qc + 1) * 512],
                        perf_mode=mybir.MatmulPerfMode.DoubleRow,
                        start=(ec == 0), stop=(ec == EC // 2 - 1))
                nc.vector.tensor_copy(
                    out=qT[:, lt, qc * 512:(qc + 1) * 512], in_=ps[:])


def _phase_a(nc, tc, mask_h, kv_loc, kv_g, v_d, v_g, S, qT, riall,
             v_sb, split):
    """Scores (fp8 DoubleRow) + mask + fused per-qt softmax."""
    NKB = SK // 512
    with tc.tile_pool(name="pa_kt", bufs=2) as pa_kt, \
         tc.tile_pool(name="pa_mi", bufs=2) as pa_mi, \
         tc.tile_pool(name="pa_mf", bufs=2) as pa_mf, \
         tc.tile_pool(name="pa_sm", bufs=4) as pa_sm, \
         tc.tile_pool(name="pa_ps", bufs=3, space="PSUM") as pa_ps:

        for kb in range(NKB):
            ksl = slice(kb * 512, (kb + 1) * 512)
            kt_blk = pa_kt.tile([P, LC, 512], F8, tag="ktb")   # 2 MB
            if split:
                g, sub = kb // 2, kb % 2
                kt_src = kv_g.ap()[g].rearrange(
                    "(c p) k -> p c k", p=P)[:, :, sub * 512:(sub + 1) * 512]
            else:
                kt_src = kv_loc.ap().rearrange(
                    "(c p) k -> p c k", p=P)[:, :, ksl]
            # gpsimd queue: keeps the scalar queue free for the Wq stream
            # and avoids head-blocking it behind the collective-gated load
            nc.gpsimd.dma_start(out=kt_blk[:], in_=kt_src)
            if kb == 0:
                # prefetch v for phase B
                if split:
                    for gv in range(2):
                        nc.gpsimd.dma_start(
                            out=v_sb[:, gv * KCH:(gv + 1) * KCH, :],
                            in_=v_g.ap()[gv].rearrange(
                                "(c p) e -> p c e", p=P))
                else:
                    nc.gpsimd.dma_start(
                        out=v_sb[:],
                        in_=v_d.ap().rearrange("(c p) e -> p c e", p=P))
            for qt in range(QT):
                ps = pa_ps.tile([P, 512], F32, tag="s")
                for lc in range(LC // 2):
                    nc.tensor.matmul(
                        ps[:], qT[:, 2 * lc:2 * lc + 2, qt * P:(qt + 1) * P],
                        kt_blk[:, 2 * lc:2 * lc + 2, :],
                        perf_mode=mybir.MatmulPerfMode.DoubleRow,
                        start=(lc == 0), stop=(lc == LC // 2 - 1))
                mi = pa_mi.tile([P, 512], I32, tag="mi")
                nc.sync.dma_start(
                    out=mi[:], in_=mask_h.ap()[qt * P:(qt + 1) * P, ksl])
                mf = pa_mf.tile([P, 512], F32, tag="mf")
                nc.vector.tensor_scalar_mul(out=mf[:], in0=mi[:], scalar1=NEG)
                nc.vector.tensor_add(out=S[:, qt, ksl], in0=ps[:], in1=mf[:])
                if kb == NKB - 1:
                    # No row-max subtraction: scaled logits are ~N(0,1)
                    # (|s|/64 < ~6), exp can't overflow f32; masked entries
                    # give exp(-1.5e28) = 0.
                    rs = pa_sm.tile([P, 1], F32, tag="rs")
                    nc.scalar.activation(
                        out=S[:, qt, :], in_=S[:, qt, :], func=AF.Exp,
                        bias=0.0, scale=INV_SQRT_L, accum_out=rs[:])
                    nc.vector.reciprocal(out=riall[:, qt:qt + 1], in_=rs[:])


def _phase_b(nc, tc, xn_d, x3_d, x3T_d, S, v_sb, riall, ident, eps_t):
    """P^T, out2 = P@V, residual, LN3, x3T spill (transposed, bf16)."""
    x3T_w = x3T_d.ap().rearrange("(c p) q -> p c q", p=P)
    with tc.tile_pool(name="pb_pt", bufs=2 * KC) as pb_pt, \
         tc.tile_pool(name="pb_x", bufs=2) as pb_x, \
         tc.tile_pool(name="x3b_pool", bufs=QT) as x3b_pool, \
         tc.tile_pool(name="pb_st", bufs=2) as pb_st, \
         tc.tile_pool(name="pb_tmp", bufs=4) as pb_tmp, \
         tc.tile_pool(name="pb_ptps", bufs=4, space="PSUM") as pb_ptps, \
         tc.tile_pool(name="pb_ps", bufs=4, space="PSUM") as pb_ps:

        def transpose_s(qt):
            pts = []
            for kc in range(KC):
                pps = pb_ptps.tile([P, P], BF16, tag="ptps")
                nc.tensor.transpose(
                    pps[:], S[:, qt, kc * P:(kc + 1) * P], ident[:])
                pt = pb_pt.tile([P, P], BF16, tag="pt")
                # alternate copy engines so the a@v chain isn't gated on DVE
                eng = nc.vector if kc % 2 == 0 else nc.scalar
                if eng is nc.scalar:
                    nc.scalar.copy(out=pt[:], in_=pps[:])
                else:
                    nc.vector.tensor_copy(out=pt[:], in_=pps[:])
                pts.append(pt)
            return pts

        x3bs = []
        pts_next = transpose_s(0)
        for qt in range(QT):
            pts = pts_next
            if qt + 1 < QT:
                pts_next = transpose_s(qt + 1)

            xn_t = pb_x.tile([P, E], F32, tag="xn")
            # gpsimd: the Act engine is busy with the softmax exp tail here
            nc.gpsimd.dma_start(
                out=xn_t[:], in_=xn_d.ap()[qt * P:(qt + 1) * P, :])
            x2 = pb_x.tile([P, E], F32, tag="x2")
            for eo in range(E // 512):
                ps = pb_ps.tile([P, 512], F32, tag="o")
                for kc in range(KC):
                    nc.tensor.matmul(
                        ps[:], pts[kc][:],
                        v_sb[:, kc, eo * 512:(eo + 1) * 512],
                        start=(kc == 0), stop=(kc == KC - 1))
                nc.vector.tensor_scalar_mul(
                    out=x2[:, eo * 512:(eo + 1) * 512], in0=ps[:],
                    scalar1=riall[:, qt:qt + 1])
            nc.vector.tensor_add(out=x2[:], in0=x2[:], in1=xn_t[:])

            x3 = pb_x.tile([P, E], F32, tag="x3")
            _layernorm_tile(nc, pb_tmp, x3[:], x2[:], eps_t)
            nc.gpsimd.dma_start(
                out=x3_d.ap()[qt * P:(qt + 1) * P, :], in_=x3[:])
            x3b = x3b_pool.tile([P, E], BF16, tag="x3b")
            nc.gpsimd.tensor_copy(out=x3b[:], in_=x3[:])
            x3bs.append(x3b)

        for qt in range(QT):    # trailing transposes: no PE head-of-line
            st = pb_st.tile([P, EC, P], BF16, tag="st")
            for ec in range(EC):
                pps = pb_ptps.tile([P, P], BF16, tag="ptps")
                nc.tensor.transpose(
                    pps[:], x3bs[qt][:, ec * P:(ec + 1) * P], ident[:])
                nc.scalar.copy(out=st[:, ec, :], in_=pps[:])
            nc.sync.dma_start(
                out=x3T_w[:, :, qt * P:(qt + 1) * P], in_=st[:])


def _phase_f(nc, tc, Win, Wout, x3_d, x3T_d, out1, hT, wout_lo, wout_hi):
    """FFN: hT = relu(Win^T @ x3T); out = hT^T @ Wout + x3."""
    wout_r = Wout.ap().rearrange("(c p) e -> p c e", p=P)
    with tc.tile_pool(name="pf_x3T", bufs=1) as pf_x3T, \
         tc.tile_pool(name="pf_w", bufs=3) as pf_w, \
         tc.tile_pool(name="pf_wf", bufs=3) as pf_wf, \
         tc.tile_pool(name="pf_wb", bufs=3) as pf_wb, \
         tc.tile_pool(name="pf_x", bufs=2) as pf_x, \
         tc.tile_pool(name="pf_o", bufs=3) as pf_o, \
         tc.tile_pool(name="pf_ps", bufs=3, space="PSUM") as pf_ps:

        x3T = pf_x3T.tile([P, EC, SQH], BF16)    # 2 MB
        nc.scalar.dma_start(
            out=x3T[:], in_=x3T_d.ap().rearrange("(c p) q -> p c q", p=P))

        for lt in range(LC):
            lsl = slice(lt * P, (lt + 1) * P)
            wi_f = pf_w.tile([P, EC, P], F32, tag="wi_f")
            nc.scalar.dma_start(
                out=wi_f[:],
                in_=Win.ap()[:, lsl].rearrange("(c p) l -> p c l", p=P))
            wi_b = pf_wb.tile([P, EC, P], BF16, tag="wi_b")
            nc.scalar.copy(out=wi_b[:], in_=wi_f[:])
            # Wout loads on the gpsimd queue, cast on DVE: keeps the
            # scalar queue free for Win and the Act engine for relu copies
            wf = pf_wf.tile([P, E], F32, tag="wo_f")
            nc.gpsimd.dma_start(out=wf[:], in_=wout_r[:, lt, :])
            wdst = wout_lo if lt < KC else wout_hi
            nc.vector.tensor_copy(out=wdst[:, lt % KC, :], in_=wf[:])
            for qc in range(SQH // 512):
                ps = pf_ps.tile([P, 512], F32, tag="h")
                for ec in range(EC):
                    nc.tensor.matmul(
                        ps[:], wi_b[:, ec, :],
                        x3T[:, ec, qc * 512:(qc + 1) * 512],
                        start=(ec == 0), stop=(ec == EC - 1))
                nc.scalar.activation(
                    out=hT[:, lt, qc * 512:(qc + 1) * 512], in_=ps[:],
                    func=AF.Relu)

        for qt in range(QT):
            x3_t = pf_x.tile([P, E], F32, tag="x3r")
            nc.sync.dma_start(
                out=x3_t[:], in_=x3_d.ap()[qt * P:(qt + 1) * P, :])
            for eo in range(E // 512):
                ps = pf_ps.tile([P, 512], F32, tag="f")
                for lc in range(LC):
                    wsrc = wout_lo if lc < KC else wout_hi
                    nc.tensor.matmul(
                        ps[:], hT[:, lc, qt * P:(qt + 1) * P],
                        wsrc[:, lc % KC, eo * 512:(eo + 1) * 512],
                        start=(lc == 0), stop=(lc == LC - 1))
                o_t = pf_o.tile([P, 512], F32, tag="o")
                nc.vector.tensor_add(
                    out=o_t[:], in0=ps[:], in1=x3_t[:, eo * 512:(eo + 1) * 512])
                nc.sync.dma_start(
                    out=out1.ap()[qt * P:(qt + 1) * P, eo * 512:(eo + 1) * 512],
                    in_=o_t[:])


def _get_compiled(phases="12vabf", sim=False, nocoll=False, split=None):
    key = (phases, sim, nocoll, SPLIT_KV if split is None else split)
    if key not in _CACHE:
        _CACHE[key] = _build(phases, sim, nocoll, split)
    return _CACHE[key]


def _check_trivial(inputs):
    for n in ("ln1_w", "ln2_w", "ln3_w"):
        if n in inputs and not np.allclose(np.asarray(inputs[n]), 1.0):
            raise NotImplementedError(f"nontrivial {n} unsupported")
    for n in ("ln1_b", "ln2_b", "ln3_b", "bq", "bk", "bv", "bin", "bout"):
        if n in inputs and not np.allclose(np.asarray(inputs[n]), 0.0):
            raise NotImplementedError(f"nontrivial {n} unsupported")


LAST_EXEC_NS = None
TRACE = False


def make_in_maps(x, y, mask, Wq, Wk, Wv, Win, Wout, split=None):
    if split is None:
        split = SPLIT_KV
    in_maps = []
    for c in range(NCORES):
        b, h = c // 2, c % 2
        ysl = y[b, h * SKH:(h + 1) * SKH] if split else y[b]
        in_maps.append({
            "x_h": np.ascontiguousarray(x[b, h * SQH:(h + 1) * SQH]),
            "y_h": np.ascontiguousarray(ysl),
            "mask_h": np.ascontiguousarray(mask[b, h * SQH:(h + 1) * SQH]),
            "Wq": Wq, "Wk": Wk, "Wv": Wv, "Win": Win, "Wout": Wout,
        })
    return in_maps


def kernel(**inputs):
    global LAST_EXEC_NS
    _check_trivial(inputs)
    x = np.ascontiguousarray(np.asarray(inputs["x"], dtype=np.float32))
    y = np.ascontiguousarray(np.asarray(inputs["y"], dtype=np.float32))
    mask = np.ascontiguousarray(np.asarray(inputs["mask"], dtype=np.int32))
    Wq = np.ascontiguousarray(np.asarray(inputs["Wq"], dtype=np.float32))
    Wk = np.ascontiguousarray(np.asarray(inputs["Wk"], dtype=np.float32))
    Wv = np.ascontiguousarray(np.asarray(inputs["Wv"], dtype=np.float32))
    Win = np.ascontiguousarray(np.asarray(inputs["Win"], dtype=np.float32))
    Wout = np.ascontiguousarray(np.asarray(inputs["Wout"], dtype=np.float32))

    nc = _get_compiled()
    in_maps = make_in_maps(x, y, mask, Wq, Wk, Wv, Win, Wout)
    last_err = None
    for attempt in range(3):
        try:
            res = run_bass_kernel_spmd(nc, in_maps,
                                       core_ids=list(range(NCORES)),
                                       trace=TRACE)
            break
        except Exception as e:   # transient device/terminal errors
            last_err = e
            import time as _time
            _time.sleep(10)
    else:
        raise last_err
    LAST_EXEC_NS = res.exec_time_ns
    outs = res.results
    o1 = np.empty((B, 2 * SQH, E), np.float32)
    yn = np.empty((B, SK, E), np.float32)
    for c in range(NCORES):
        b, h = c // 2, c % 2
        o1[b, h * SQH:(h + 1) * SQH] = outs[c]["out1"]
        if SPLIT_KV:
            yn[b, h * SKH:(h + 1) * SKH] = outs[c]["yn_out"]
        elif h == 0:
            yn[b] = outs[c]["yn_out"]
    return o1, yn
